# revision 27
# baseline (speedup 1.0000x reference)
"""Causal self-attention (B=16, S=2048, D=512) on 8 Trainium2 NeuronCores.

Strategy: data-parallel over batch (2 sequences per core), QKV weights
replicated. Per sequence everything is computed in transposed layouts so no
on-device transposes are needed:

  host prep:  xT = x^T per sequence [D, S] (bf16);  wqT = Wq^T/sqrt(D);
              wkT = Wk^T;  wvT = Wv^T (bf16);  bq' = bq/sqrt(D);
              key-pad bias (0/-1e30);  query mask (1/0) as floats.

  device (per sequence, KB = valid 128-blocks for the slot):
    QT[d,s]  = wqT^T.slices @ xT   (+bq' on DVE eviction, bf16 out)
    KT[d,s]  = wkT^T.slices @ xT   (+bk on DVE eviction, bf16 out)
    V[s,d]   = xT^T.slices @ wvT   (+bv via rank-1 ones matmul, bf16 out)
    per q-chunk qc (512 queries), software-pipelined:
      D(qc):   dacc = DVE f32 tree-sum of exp tiles; pden[1,q] = ones^T @
               dacc (PE); dcp = copy(pden) (ACT) -> reshape DMA [4,128]
      S(qc+1): scoresT[k,q] = KT.T @ QT (bf16, diagonal blocks trimmed to
               their valid right part); expT = Exp(scores + keybias[k]) on
               ACT; diagonal blocks multiplied by a 0/1 lower-triangle bf16
               tile on DVE; left-of-diagonal regions zero-filled on GPSIMD.
      T(qc):   PE transpose [4,128]->[128,4]; scl = qmask/(den+eps) (DVE);
               emitted after the first two out-MM groups so the ACT+DMA
               denominator chain is fully hidden.
      O(qc):   out_un[q,d] = expT.slices^T @ V  (accumulate over k blocks)
               out = out_un * scl[q]  (ACT Identity with per-partition scale)

softmax equivalence: exp without max-subtraction, masked entries exactly 0;
rows with a padded query are zeroed by qmask (matches reference's
post-softmax zeroing). Blocks of rows past KB are zero-filled by DMA.

All attention matmuls and the projections run in bfloat16 (f32r streams
slower on this silicon: measured 328 vs 281 ns per [128x128]x[128x512] MM);
accumulation stays fp32 in PSUM. rel err ~5e-3, gate is 2e-2.
"""

import contextlib
from types import SimpleNamespace

import numpy as np

import concourse.bacc as bacc
import concourse.mybir as mybir
from concourse.tile import TileContext
from concourse.bass_utils import run_bass_kernel_spmd

B, S, D = 16, 2048, 512
N_CORES = 8
BPC = B // N_CORES          # sequences per core
P = 128                     # partition dim
W = 512                     # matmul moving width (one PSUM bank of fp32)
DC = D // P                 # 4 contraction chunks of 128 over D
SB = S // P                 # 16 blocks of 128 over S (k/q/s blocks)
QC = S // W                 # 4 query chunks of 512
WB = W // P                 # 4 q-blocks per chunk
NEG = -1.0e30
EPS = 1.0e-30

f32 = mybir.dt.float32
f32r = mybir.dt.float32r
bf16 = mybir.dt.bfloat16


def _emit_scores(e, g, qc):
    """scoresT -> exp (ACT) -> causal tri-mult (DVE) + left memset (POOL)."""
    nc = e.nc
    tiles = []
    for kb in range(g.kmax_of(qc)):
        j0 = kb - qc * WB
        cs = max(0, j0) * P  # valid col start (diagonal trim)
        pscore = e.psp.tile([P, W], f32, tag="ps")
        for dc in range(DC):
            nc.tensor.matmul(
                pscore[:, cs:],
                g.kT[dc][:, kb * P:(kb + 1) * P],
                g.qT[dc][:, qc * W + cs:(qc + 1) * W],
                start=(dc == 0),
                stop=(dc == DC - 1),
            )
        et = e.ep.tile([P, W], bf16, tag="et")
        nc.scalar.activation(
            et[:, cs:],
            pscore[:, cs:],
            mybir.ActivationFunctionType.Exp,
            bias=g.kbias_t[:, kb:kb + 1],
            scale=1.0,
        )
        if j0 >= 0:
            nc.vector.tensor_tensor(
                et[:, cs:cs + P], et[:, cs:cs + P],
                e.tri_t[:], op=mybir.AluOpType.mult,
            )
        tiles.append(et)
    return tiles


def _emit_out_mms(e, g, qc, jq, exp_tiles):
    nc = e.nc
    qb = qc * WB + jq
    pout = e.pop.tile([P, W], f32, tag="po")
    for kb in range(qb + 1):
        nc.tensor.matmul(
            pout[:],
            exp_tiles[kb][:, jq * P:(jq + 1) * P],
            g.vv[kb][:],
            start=(kb == 0),
            stop=(kb == qb),
        )
    return pout


def _emit_out_evict(e, g, qc, jq, pout, scl):
    nc = e.nc
    qb = qc * WB + jq
    ot = e.op_.tile([P, W], f32, tag="outs")
    nc.scalar.activation(
        ot[:],
        pout[:],
        mybir.ActivationFunctionType.Identity,
        bias=0.0,
        scale=scl[:, jq:jq + 1],
    )
    nc.sync.dma_start(
        out=e.out_d[g.seq, qb * P:(qb + 1) * P, :],
        in_=ot[:],
    )


def _emit_attention(e, g):
    """Software-pipelined q-chunk loop for one sequence."""
    nc = e.nc
    SCcap = g.SCcap
    exp_tiles = [None] * SCcap
    exp_tiles[0] = _emit_scores(e, g, 0)
    for qc in range(SCcap):
        kmax = g.kmax_of(qc)
        # D(qc): denominators via PE psum accumulation over k blocks
        # (diagonal tiles contribute only their computed column ranges)
        pden = e.pdp.tile([1, W], f32, tag="pden")
        for kb in range(kmax):
            j0 = kb - qc * WB
            cs = max(0, j0) * P
            nc.tensor.matmul(
                pden[0:1, cs:],
                e.onesb_t[:],
                exp_tiles[qc][kb][:, cs:],
                start=(kb == 0),
                stop=(kb == kmax - 1),
            )
        dcp = e.mp.tile([1, W], f32, tag="dcp")
        nc.scalar.copy(dcp[:], pden[:])
        den4 = e.mp.tile([WB, P], f32, tag="den4")
        nc.sync.dma_start(out=den4[:], in_=dcp[0:1, :])

        # S(qc+1): next chunk's scores keep PE busy while the denominator
        # chain goes ACT -> DMA
        if qc + 1 < SCcap:
            exp_tiles[qc + 1] = _emit_scores(e, g, qc + 1)

        # T(qc): transpose + per-q scale
        pdt = e.ptp.tile([P, WB], f32, tag="pdt")
        nc.tensor.transpose(pdt[:], den4[:], e.ident[:WB, :WB])
        scl = e.mp.tile([P, WB], f32, tag="scl")
        nc.vector.tensor_scalar_add(scl[:], pdt[:], EPS)
        nc.vector.reciprocal(scl[:], scl[:])
        nc.vector.tensor_tensor(
            scl[:], scl[:], g.qmask_t[:, qc * WB:(qc + 1) * WB],
            op=mybir.AluOpType.mult,
        )

        # O(qc): out groups + evictions
        for jq in range(WB):
            qb = qc * WB + jq
            if qb >= g.KB:
                continue  # all-padded query rows: zero-filled
            pout = _emit_out_mms(e, g, qc, jq, exp_tiles[qc])
            _emit_out_evict(e, g, qc, jq, pout, scl)
        exp_tiles[qc] = None  # release slots


def _emit_sequence(e, seq, KB, weights):
    nc = e.nc
    wq, wk, wv = weights
    KCOLS = KB * P
    SCcap = -(-KB // WB)

    kbias_t = e.mp.tile([P, SB], f32, tag="kbias")
    nc.sync.dma_start(
        out=kbias_t[:], in_=e.kbias_d[seq].rearrange("(n p) -> p n", p=P))
    qmask_t = e.mp.tile([P, SB], f32, tag="qmask")
    nc.sync.dma_start(
        out=qmask_t[:], in_=e.qmask_d[seq].rearrange("(n p) -> p n", p=P))

    xt = []
    for c in range(DC):
        t = e.xp.tile([P, S], bf16, tag="xt")
        nc.sync.dma_start(
            out=t[:, :KCOLS], in_=e.xT_d[seq, c * P:(c + 1) * P, :KCOLS])
        xt.append(t)

    # ---- projections (sc-major so attention can start early) ----
    qT, kT = [], []
    for db in range(DC):
        tq = e.qp.tile([P, S], bf16, tag="qt")
        tk = e.kp.tile([P, S], bf16, tag="kt")
        qT.append(tq)
        kT.append(tk)
    for sc in range(SCcap):
        w = min(W, KCOLS - sc * W)
        for proj, wmat, qkT, bias in (("q", wq, qT, e.bq_t), ("k", wk, kT, e.bk_t)):
            for db in range(DC):
                pq = e.pp.tile([P, W], f32, tag="pp")
                for c in range(DC):
                    nc.tensor.matmul(
                        pq[:, :w],
                        wmat[c][:, db * P:(db + 1) * P],
                        xt[c][:, sc * W:sc * W + w],
                        start=(c == 0),
                        stop=(c == DC - 1),
                    )
                if proj == "q":
                    # Q evictions on DVE, K evictions on ACT: splits the
                    # PSUM->SBUF eviction load so neither engine gates the
                    # projection matmul stream
                    nc.vector.tensor_scalar_add(
                        qkT[db][:, sc * W:sc * W + w],
                        pq[:, :w],
                        bias[:, db:db + 1],
                    )
                else:
                    nc.scalar.activation(
                        qkT[db][:, sc * W:sc * W + w],
                        pq[:, :w],
                        mybir.ActivationFunctionType.Identity,
                        bias=bias[:, db:db + 1],
                        scale=1.0,
                    )
    # V: [s_block 128, d 512], bias added during DVE eviction
    vv = []
    for sb_ in range(KB):
        pv = e.pp.tile([P, W], f32, tag="pp")
        for c in range(DC):
            nc.tensor.matmul(
                pv[:],
                xt[c][:, sb_ * P:(sb_ + 1) * P],
                wv[c][:],
                start=(c == 0),
                stop=(c == DC - 1),
            )
        tv = e.vp.tile([P, W], bf16, tag="vv")
        nc.vector.tensor_add(tv[:], pv[:], e.bvb_t[:])
        vv.append(tv)

    g = SimpleNamespace(
        seq=seq, KB=KB, KCOLS=KCOLS, SCcap=SCcap,
        kbias_t=kbias_t, qmask_t=qmask_t, qT=qT, kT=kT, vv=vv,
        kmax_of=lambda qc: min((qc + 1) * WB, KB),
    )
    _emit_attention(e, g)

    # rows in blocks >= KB are entirely padded queries: zero
    for qb in range(KB, SB):
        nc.sync.dma_start(
            out=e.out_d[seq, qb * P:(qb + 1) * P, :], in_=e.zt[:])


def _emit_iteration(e, slot_caps):
    nc = e.nc
    # weights, once per iteration
    weights = []
    for wd in (e.wqT_d, e.wkT_d, e.wvT_d):
        lst = []
        for c in range(DC):
            t = e.wp.tile([P, W], bf16, tag="wgt")
            nc.sync.dma_start(out=t[:], in_=wd[c * P:(c + 1) * P, :])
            lst.append(t)
        weights.append(lst)
    for seq in range(BPC):
        _emit_sequence(e, seq, slot_caps[seq], weights)


def build_nc(repeat: int = 1, loop: bool = False, slot_caps=(SB, SB),
             staggered: bool = False):
    """slot_caps[s] = number of 128-blocks of valid (non-padded) positions for
    sequence slot s on every core (program-wide). Blocks beyond the cap hold
    only padded positions: their keys contribute exactly 0 (key bias) and
    their query rows are exactly 0 in the reference (query mask), so skipping
    them and zero-filling the output rows is exact for any mask."""
    nc = bacc.Bacc()

    e = SimpleNamespace(nc=nc)
    e.xT_d = nc.declare_dram_parameter("xT", [BPC, D, S], bf16, isOutput=False)
    e.wqT_d = nc.declare_dram_parameter("wqT", [D, D], bf16, isOutput=False)
    e.wkT_d = nc.declare_dram_parameter("wkT", [D, D], bf16, isOutput=False)
    e.wvT_d = nc.declare_dram_parameter("wvT", [D, D], bf16, isOutput=False)
    bq_d = nc.declare_dram_parameter("bq", [D], f32, isOutput=False)
    bk_d = nc.declare_dram_parameter("bk", [D], f32, isOutput=False)
    bv_d = nc.declare_dram_parameter("bv", [1, D], f32r, isOutput=False)
    e.kbias_d = nc.declare_dram_parameter("kbias", [BPC, S], f32, isOutput=False)
    e.qmask_d = nc.declare_dram_parameter("qmask", [BPC, S], f32, isOutput=False)
    onesr_d = nc.declare_dram_parameter("onesr", [1, P], f32r, isOutput=False)
    ones_d = nc.declare_dram_parameter("ones", [P, 1], f32r, isOutput=False)
    e.out_d = nc.declare_dram_parameter("out", [BPC, S, D], f32, isOutput=True)

    with TileContext(nc) as tc:
        with (
            tc.tile_pool(name="persist", bufs=1) as pers,
            tc.tile_pool(name="xt", bufs=6) as xp,
            tc.tile_pool(name="qt", bufs=2 * DC) as qp,
            tc.tile_pool(name="kt", bufs=2 * DC) as kp,
            tc.tile_pool(name="vv", bufs=20) as vp,
            tc.tile_pool(name="wgt", bufs=3 * DC) as wp,
            tc.tile_pool(name="et", bufs=28) as ep,
            tc.tile_pool(name="outs", bufs=3) as op_,
            tc.tile_pool(name="misc", bufs=2) as mp,
            tc.tile_pool(name="pp", bufs=2, space="PSUM") as pp,
            tc.tile_pool(name="ps", bufs=2, space="PSUM") as psp,
            tc.tile_pool(name="pden", bufs=1, space="PSUM") as pdp,
            tc.tile_pool(name="pdt", bufs=1, space="PSUM") as ptp,
            tc.tile_pool(name="po", bufs=2, space="PSUM") as pop,
        ):
            e.xp, e.qp, e.kp, e.vp, e.wp, e.ep = xp, qp, kp, vp, wp, ep
            e.op_, e.mp, e.pp, e.psp, e.pdp, e.ptp, e.pop = op_, mp, pp, psp, pdp, ptp, pop

            # ---- persistent setup (once) ----
            onesr_t = pers.tile([1, P], f32r, tag="onesr")
            nc.sync.dma_start(out=onesr_t[:], in_=onesr_d[:])
            bv_t = pers.tile([1, D], f32r, tag="bv")
            nc.sync.dma_start(out=bv_t[:], in_=bv_d[:])
            e.bq_t = pers.tile([P, DC], f32, tag="bq")
            nc.sync.dma_start(out=e.bq_t[:], in_=bq_d.rearrange("(n p) -> p n", p=P))
            e.bk_t = pers.tile([P, DC], f32, tag="bk")
            nc.sync.dma_start(out=e.bk_t[:], in_=bk_d.rearrange("(n p) -> p n", p=P))

            # bf16 ones column for the denominator reduction matmuls
            # (stationary dtype matches the bf16 exp-tile moving operand)
            e.onesb_t = pers.tile([P, 1], bf16, tag="onesb")
            nc.gpsimd.memset(e.onesb_t[:], 1.0)

            # 0/1 lower-triangle tile: tri[k, q] = 1 if q >= k else 0
            e.tri_t = pers.tile([P, P], bf16, tag="tri")
            nc.gpsimd.memset(e.tri_t[:], 1.0)
            nc.gpsimd.affine_select(
                out=e.tri_t[:], in_=e.tri_t[:],
                compare_op=mybir.AluOpType.is_ge, fill=0.0,
                base=0, pattern=[[1, P]], channel_multiplier=-1,
            )

            # identity for PE-mode transpose of the [4,128] denominator strip
            e.ident = pers.tile([P, P], f32, tag="ident")
            nc.gpsimd.memset(e.ident[:], 0.0)
            nc.gpsimd.affine_select(
                out=e.ident[:], in_=e.ident[:],
                compare_op=mybir.AluOpType.not_equal, fill=1.0,
                base=0, pattern=[[-1, P]], channel_multiplier=1,
            )

            # bv broadcast to all partitions via one rank-1 matmul (ones x bv)
            pbv = pp.tile([P, W], f32, tag="pp")
            nc.tensor.matmul(pbv[:], onesr_t[:], bv_t[:], start=True, stop=True)
            e.bvb_t = pers.tile([P, W], bf16, tag="bvb")
            nc.vector.tensor_copy(e.bvb_t[:], pbv[:])

            # zero tile for output rows beyond a slot's valid-block cap
            e.zt = pers.tile([P, W], f32, tag="zt")
            nc.gpsimd.memset(e.zt[:], 0.0)

            if loop:
                rep_ctx = tc.For_i(
                    0, repeat, 1,
                    hint_engines=(mybir.EngineType.PE,),
                    staggered_reset=staggered,
                )
            else:
                rep_ctx = contextlib.nullcontext(0)
            with rep_ctx:
                for _rep in range(1 if loop else repeat):
                    _emit_iteration(e, slot_caps)
    nc.finalize()
    return nc


def prep_inputs(x, Wq, bq, Wk, bk, Wv, bv, padding_mask):
    """Host-side layout prep + sharding. Returns per-core in_maps."""
    import ml_dtypes
    bf = ml_dtypes.bfloat16
    x = np.asarray(x, dtype=np.float32)
    pad = np.asarray(padding_mask).astype(bool)
    sc = 1.0 / np.sqrt(np.float32(D))
    wqT = np.ascontiguousarray((np.asarray(Wq, np.float32).T * sc).astype(bf))
    wkT = np.ascontiguousarray(np.asarray(Wk, np.float32).T.astype(bf))
    wvT = np.ascontiguousarray(np.asarray(Wv, np.float32).T.astype(bf))
    bq_s = (np.asarray(bq, np.float32) * sc).astype(np.float32)
    bk_a = np.asarray(bk, np.float32)
    bv_a = np.asarray(bv, np.float32).reshape(1, D)
    kbias = np.where(pad, np.float32(NEG), np.float32(0.0)).astype(np.float32)
    qmask = np.where(pad, np.float32(0.0), np.float32(1.0)).astype(np.float32)
    onesr = np.ones((1, P), dtype=np.float32)
    ones = np.ones((P, 1), dtype=np.float32)

    xT = np.ascontiguousarray(x.transpose(0, 2, 1).astype(bf))  # [B, D, S]

    # per-seq valid-block cap from the actual mask (exact for any mask):
    # blocks after the last non-padded position hold only padded positions
    valid = ~pad
    caps = np.zeros(B, dtype=np.int64)
    for b in range(B):
        idx = np.nonzero(valid[b])[0]
        caps[b] = 0 if idx.size == 0 else int(np.ceil((idx[-1] + 1) / P))
    order = np.argsort(-caps, kind="stable")  # descending cap
    # core i runs (slot0 = order[2*N_CORES-1-i] short, slot1 = order[i] long)
    perm = []
    for i in range(N_CORES):
        perm.extend([int(order[B - 1 - i]), int(order[i])])
    slot_caps = (int(caps[order[N_CORES]]), int(caps[order[0]]))

    in_maps = []
    for i in range(N_CORES):
        sel = [perm[2 * i], perm[2 * i + 1]]
        in_maps.append({
            "xT": np.ascontiguousarray(xT[sel]),
            "wqT": wqT, "wkT": wkT, "wvT": wvT,
            "bq": bq_s, "bk": bk_a, "bv": bv_a,
            "kbias": np.ascontiguousarray(kbias[sel]),
            "qmask": np.ascontiguousarray(qmask[sel]),
            "onesr": onesr, "ones": ones,
        })
    return in_maps, perm, slot_caps


_NC_CACHE = {}
STAGGERED = False  # loop back-edge mode (A/B toggle for experiments)


def get_nc(repeat: int = 1, loop: bool = False, slot_caps=(SB, SB)):
    key = (repeat, loop, slot_caps, STAGGERED)
    if key not in _NC_CACHE:
        _NC_CACHE[key] = build_nc(repeat, loop, slot_caps, staggered=STAGGERED)
    return _NC_CACHE[key]


def kernel(x, Wq, bq, Wk, bk, Wv, bv, padding_mask):
    in_maps, perm, slot_caps = prep_inputs(
        x, Wq, bq, Wk, bk, Wv, bv, padding_mask)
    nc = get_nc(1, slot_caps=slot_caps)
    r = run_bass_kernel_spmd(nc, in_maps, list(range(N_CORES)))
    out = np.empty((B, S, D), dtype=np.float32)
    for j, orig in enumerate(perm):
        out[orig] = r.results[j // BPC]["out"][j % BPC]
    return out


# revision 31
# speedup vs baseline: 1.1790x; 1.1790x over previous
"""Causal self-attention (B=16, S=2048, D=512) on 8 Trainium2 NeuronCores.

Strategy: data-parallel over batch (2 sequences per core), QKV weights
replicated. Per sequence everything is computed in transposed layouts so no
on-device transposes are needed:

  host prep:  xT = x^T per sequence [D, S] (bf16);  wqT = Wq^T/sqrt(D);
              wkT = Wk^T;  wvT = Wv^T (bf16);  bq' = bq/sqrt(D);
              key-pad bias (0/-1e30);  query mask (1/0) as floats.

  device (per sequence, KB = valid 128-blocks for the slot):
    QT[d,s]  = wqT^T.slices @ xT   (+bq' on DVE eviction, bf16 out)
    KT[d,s]  = wkT^T.slices @ xT   (+bk on DVE eviction, bf16 out)
    V[s,d]   = xT^T.slices @ wvT   (+bv via rank-1 ones matmul, bf16 out)
    per q-chunk qc (512 queries), software-pipelined:
      D(qc):   dacc = DVE f32 tree-sum of exp tiles; pden[1,q] = ones^T @
               dacc (PE); dcp = copy(pden) (ACT) -> reshape DMA [4,128]
      S(qc+1): scoresT[k,q] = KT.T @ QT (bf16, diagonal blocks trimmed to
               their valid right part); expT = Exp(scores + keybias[k]) on
               ACT; diagonal blocks multiplied by a 0/1 lower-triangle bf16
               tile on DVE; left-of-diagonal regions zero-filled on GPSIMD.
      T(qc):   PE transpose [4,128]->[128,4]; scl = qmask/(den+eps) (DVE);
               emitted after the first two out-MM groups so the ACT+DMA
               denominator chain is fully hidden.
      O(qc):   out_un[q,d] = expT.slices^T @ V  (accumulate over k blocks)
               out = out_un * scl[q]  (ACT Identity with per-partition scale)

softmax equivalence: exp without max-subtraction, masked entries exactly 0;
rows with a padded query are zeroed by qmask (matches reference's
post-softmax zeroing). Blocks of rows past KB are zero-filled by DMA.

All attention matmuls and the projections run in bfloat16 (f32r streams
slower on this silicon: measured 328 vs 281 ns per [128x128]x[128x512] MM);
accumulation stays fp32 in PSUM. rel err ~5e-3, gate is 2e-2.
"""

import contextlib
from types import SimpleNamespace

import numpy as np

import concourse.bacc as bacc
import concourse.mybir as mybir
from concourse.tile import TileContext
from concourse.bass_utils import run_bass_kernel_spmd

B, S, D = 16, 2048, 512
N_CORES = 8
BPC = B // N_CORES          # sequences per core
P = 128                     # partition dim
W = 512                     # matmul moving width (one PSUM bank of fp32)
DC = D // P                 # 4 contraction chunks of 128 over D
SB = S // P                 # 16 blocks of 128 over S (k/q/s blocks)
QC = S // W                 # 4 query chunks of 512
WB = W // P                 # 4 q-blocks per chunk
NEG = -1.0e30
EPS = 1.0e-30

f32 = mybir.dt.float32
f32r = mybir.dt.float32r
bf16 = mybir.dt.bfloat16


def _emit_scores(e, g, qc):
    """scoresT -> exp (ACT) -> causal tri-mult (DVE) + left memset (POOL)."""
    nc = e.nc
    tiles = []
    for kb in range(g.kmax_of(qc)):
        j0 = kb - qc * WB
        cs = max(0, j0) * P  # valid col start (diagonal trim)
        pscore = e.psp.tile([P, W], f32, tag="ps")
        for dc in range(DC):
            nc.tensor.matmul(
                pscore[:, cs:],
                g.kT[dc][:, kb * P:(kb + 1) * P],
                g.qT[dc][:, qc * W + cs:(qc + 1) * W],
                start=(dc == 0),
                stop=(dc == DC - 1),
            )
        et = e.ep.tile([P, W], bf16, tag="et")
        nc.scalar.activation(
            et[:, cs:],
            pscore[:, cs:],
            mybir.ActivationFunctionType.Exp,
            bias=g.kbias_t[:, kb:kb + 1],
            scale=1.0,
        )
        if j0 >= 0:
            nc.vector.tensor_tensor(
                et[:, cs:cs + P], et[:, cs:cs + P],
                e.tri_t[:], op=mybir.AluOpType.mult,
            )
        tiles.append(et)
    return tiles


def _emit_out_mms(e, g, qc, jq, exp_tiles):
    nc = e.nc
    qb = qc * WB + jq
    pout = e.pop.tile([P, W], f32, tag="po")
    for kb in range(qb + 1):
        nc.tensor.matmul(
            pout[:],
            exp_tiles[kb][:, jq * P:(jq + 1) * P],
            g.vv[kb][:],
            start=(kb == 0),
            stop=(kb == qb),
        )
    return pout


def _emit_out_evict(e, g, qc, jq, pout, scl):
    nc = e.nc
    qb = qc * WB + jq
    ot = e.op_.tile([P, W], f32, tag="outs")
    nc.scalar.activation(
        ot[:],
        pout[:],
        mybir.ActivationFunctionType.Identity,
        bias=0.0,
        scale=scl[:, jq:jq + 1],
    )
    nc.sync.dma_start(
        out=e.out_d[g.seq, qb * P:(qb + 1) * P, :],
        in_=ot[:],
    )


def _emit_attention(e, g):
    """Software-pipelined q-chunk loop for one sequence."""
    nc = e.nc
    SCcap = g.SCcap
    exp_tiles = [None] * SCcap
    exp_tiles[0] = _emit_scores(e, g, 0)
    for qc in range(SCcap):
        kmax = g.kmax_of(qc)
        # D(qc): denominators via PE psum accumulation over k blocks
        # (diagonal tiles contribute only their computed column ranges)
        pden = e.pdp.tile([1, W], f32, tag="pden")
        for kb in range(kmax):
            j0 = kb - qc * WB
            cs = max(0, j0) * P
            nc.tensor.matmul(
                pden[0:1, cs:],
                e.onesb_t[:],
                exp_tiles[qc][kb][:, cs:],
                start=(kb == 0),
                stop=(kb == kmax - 1),
            )
        dcp = e.mp.tile([1, W], f32, tag="dcp")
        nc.scalar.copy(dcp[:], pden[:])
        den4 = e.mp.tile([WB, P], f32, tag="den4")
        nc.sync.dma_start(out=den4[:], in_=dcp[0:1, :])

        # S(qc+1): next chunk's scores keep PE busy while the denominator
        # chain goes ACT -> DMA
        if qc + 1 < SCcap:
            exp_tiles[qc + 1] = _emit_scores(e, g, qc + 1)

        # T(qc): transpose + per-q scale
        pdt = e.ptp.tile([P, WB], f32, tag="pdt")
        nc.tensor.transpose(pdt[:], den4[:], e.ident[:WB, :WB])
        scl = e.mp.tile([P, WB], f32, tag="scl")
        nc.vector.tensor_scalar_add(scl[:], pdt[:], EPS)
        nc.vector.reciprocal(scl[:], scl[:])
        nc.vector.tensor_tensor(
            scl[:], scl[:], g.qmask_t[:, qc * WB:(qc + 1) * WB],
            op=mybir.AluOpType.mult,
        )

        # O(qc): out groups + evictions
        for jq in range(WB):
            qb = qc * WB + jq
            if qb >= g.KB:
                continue  # all-padded query rows: zero-filled
            pout = _emit_out_mms(e, g, qc, jq, exp_tiles[qc])
            _emit_out_evict(e, g, qc, jq, pout, scl)
        exp_tiles[qc] = None  # release slots


def _emit_sequence(e, seq, KB, weights):
    nc = e.nc
    wq, wk, wv = weights
    KCOLS = KB * P
    SCcap = -(-KB // WB)

    kbias_t = e.mp.tile([P, SB], f32, tag="kbias")
    nc.sync.dma_start(
        out=kbias_t[:], in_=e.kbias_d[seq].rearrange("(n p) -> p n", p=P))
    qmask_t = e.mp.tile([P, SB], f32, tag="qmask")
    nc.sync.dma_start(
        out=qmask_t[:], in_=e.qmask_d[seq].rearrange("(n p) -> p n", p=P))

    xt = []
    for c in range(DC):
        t = e.xp.tile([P, S], bf16, tag="xt")
        nc.sync.dma_start(
            out=t[:, :KCOLS], in_=e.xT_d[seq, c * P:(c + 1) * P, :KCOLS])
        xt.append(t)

    # ---- projections (sc-major so attention can start early) ----
    qT, kT = [], []
    for db in range(DC):
        tq = e.qp.tile([P, S], bf16, tag="qt")
        tk = e.kp.tile([P, S], bf16, tag="kt")
        qT.append(tq)
        kT.append(tk)
    for sc in range(SCcap):
        w = min(W, KCOLS - sc * W)
        for proj, wmat, qkT, bias in (("q", wq, qT, e.bq_t), ("k", wk, kT, e.bk_t)):
            for db in range(DC):
                pq = e.pp.tile([P, W], f32, tag="pp")
                for c in range(DC):
                    nc.tensor.matmul(
                        pq[:, :w],
                        wmat[c][:, db * P:(db + 1) * P],
                        xt[c][:, sc * W:sc * W + w],
                        start=(c == 0),
                        stop=(c == DC - 1),
                    )
                if proj == "q" or not e.k_on_act:
                    nc.vector.tensor_scalar_add(
                        qkT[db][:, sc * W:sc * W + w],
                        pq[:, :w],
                        bias[:, db:db + 1],
                    )
                else:
                    # K evictions on ACT: splits the PSUM->SBUF eviction
                    # load so neither engine gates the projection stream
                    nc.scalar.activation(
                        qkT[db][:, sc * W:sc * W + w],
                        pq[:, :w],
                        mybir.ActivationFunctionType.Identity,
                        bias=bias[:, db:db + 1],
                        scale=1.0,
                    )
    # V: [s_block 128, d 512], bias added during DVE eviction
    vv = []
    for sb_ in range(KB):
        pv = e.pp.tile([P, W], f32, tag="pp")
        for c in range(DC):
            nc.tensor.matmul(
                pv[:],
                xt[c][:, sb_ * P:(sb_ + 1) * P],
                wv[c][:],
                start=(c == 0),
                stop=(c == DC - 1),
            )
        tv = e.vp.tile([P, W], bf16, tag="vv")
        nc.vector.tensor_add(tv[:], pv[:], e.bvb_t[:])
        vv.append(tv)

    g = SimpleNamespace(
        seq=seq, KB=KB, KCOLS=KCOLS, SCcap=SCcap,
        kbias_t=kbias_t, qmask_t=qmask_t, qT=qT, kT=kT, vv=vv,
        kmax_of=lambda qc: min((qc + 1) * WB, KB),
    )
    _emit_attention(e, g)

    # rows in blocks >= KB are entirely padded queries: zero
    for qb in range(KB, SB):
        nc.sync.dma_start(
            out=e.out_d[seq, qb * P:(qb + 1) * P, :], in_=e.zt[:])


def _emit_iteration(e, slot_caps):
    nc = e.nc
    # weights, once per iteration
    weights = []
    for wd in (e.wqT_d, e.wkT_d, e.wvT_d):
        lst = []
        for c in range(DC):
            t = e.wp.tile([P, W], bf16, tag="wgt")
            nc.sync.dma_start(out=t[:], in_=wd[c * P:(c + 1) * P, :])
            lst.append(t)
        weights.append(lst)
    for seq in range(BPC):
        _emit_sequence(e, seq, slot_caps[seq], weights)


def build_nc(repeat: int = 1, loop: bool = False, slot_caps=(SB, SB),
             staggered: bool = False, k_on_act: bool = False):
    """slot_caps[s] = number of 128-blocks of valid (non-padded) positions for
    sequence slot s on every core (program-wide). Blocks beyond the cap hold
    only padded positions: their keys contribute exactly 0 (key bias) and
    their query rows are exactly 0 in the reference (query mask), so skipping
    them and zero-filling the output rows is exact for any mask."""
    nc = bacc.Bacc()

    e = SimpleNamespace(nc=nc, k_on_act=k_on_act)
    e.xT_d = nc.declare_dram_parameter("xT", [BPC, D, S], bf16, isOutput=False)
    e.wqT_d = nc.declare_dram_parameter("wqT", [D, D], bf16, isOutput=False)
    e.wkT_d = nc.declare_dram_parameter("wkT", [D, D], bf16, isOutput=False)
    e.wvT_d = nc.declare_dram_parameter("wvT", [D, D], bf16, isOutput=False)
    bq_d = nc.declare_dram_parameter("bq", [D], f32, isOutput=False)
    bk_d = nc.declare_dram_parameter("bk", [D], f32, isOutput=False)
    bv_d = nc.declare_dram_parameter("bv", [1, D], f32r, isOutput=False)
    e.kbias_d = nc.declare_dram_parameter("kbias", [BPC, S], f32, isOutput=False)
    e.qmask_d = nc.declare_dram_parameter("qmask", [BPC, S], f32, isOutput=False)
    onesr_d = nc.declare_dram_parameter("onesr", [1, P], f32r, isOutput=False)
    ones_d = nc.declare_dram_parameter("ones", [P, 1], f32r, isOutput=False)
    e.out_d = nc.declare_dram_parameter("out", [BPC, S, D], f32, isOutput=True)

    with TileContext(nc) as tc:
        with (
            tc.tile_pool(name="persist", bufs=1) as pers,
            tc.tile_pool(name="xt", bufs=6) as xp,
            tc.tile_pool(name="qt", bufs=2 * DC) as qp,
            tc.tile_pool(name="kt", bufs=2 * DC) as kp,
            tc.tile_pool(name="vv", bufs=20) as vp,
            tc.tile_pool(name="wgt", bufs=3 * DC) as wp,
            tc.tile_pool(name="et", bufs=28) as ep,
            tc.tile_pool(name="outs", bufs=3) as op_,
            tc.tile_pool(name="misc", bufs=2) as mp,
            tc.tile_pool(name="pp", bufs=2, space="PSUM") as pp,
            tc.tile_pool(name="ps", bufs=2, space="PSUM") as psp,
            tc.tile_pool(name="pden", bufs=1, space="PSUM") as pdp,
            tc.tile_pool(name="pdt", bufs=1, space="PSUM") as ptp,
            tc.tile_pool(name="po", bufs=2, space="PSUM") as pop,
        ):
            e.xp, e.qp, e.kp, e.vp, e.wp, e.ep = xp, qp, kp, vp, wp, ep
            e.op_, e.mp, e.pp, e.psp, e.pdp, e.ptp, e.pop = op_, mp, pp, psp, pdp, ptp, pop

            # ---- persistent setup (once) ----
            onesr_t = pers.tile([1, P], f32r, tag="onesr")
            nc.sync.dma_start(out=onesr_t[:], in_=onesr_d[:])
            bv_t = pers.tile([1, D], f32r, tag="bv")
            nc.sync.dma_start(out=bv_t[:], in_=bv_d[:])
            e.bq_t = pers.tile([P, DC], f32, tag="bq")
            nc.sync.dma_start(out=e.bq_t[:], in_=bq_d.rearrange("(n p) -> p n", p=P))
            e.bk_t = pers.tile([P, DC], f32, tag="bk")
            nc.sync.dma_start(out=e.bk_t[:], in_=bk_d.rearrange("(n p) -> p n", p=P))

            # bf16 ones column for the denominator reduction matmuls
            # (stationary dtype matches the bf16 exp-tile moving operand)
            e.onesb_t = pers.tile([P, 1], bf16, tag="onesb")
            nc.gpsimd.memset(e.onesb_t[:], 1.0)

            # 0/1 lower-triangle tile: tri[k, q] = 1 if q >= k else 0
            e.tri_t = pers.tile([P, P], bf16, tag="tri")
            nc.gpsimd.memset(e.tri_t[:], 1.0)
            nc.gpsimd.affine_select(
                out=e.tri_t[:], in_=e.tri_t[:],
                compare_op=mybir.AluOpType.is_ge, fill=0.0,
                base=0, pattern=[[1, P]], channel_multiplier=-1,
            )

            # identity for PE-mode transpose of the [4,128] denominator strip
            e.ident = pers.tile([P, P], f32, tag="ident")
            nc.gpsimd.memset(e.ident[:], 0.0)
            nc.gpsimd.affine_select(
                out=e.ident[:], in_=e.ident[:],
                compare_op=mybir.AluOpType.not_equal, fill=1.0,
                base=0, pattern=[[-1, P]], channel_multiplier=1,
            )

            # bv broadcast to all partitions via one rank-1 matmul (ones x bv)
            pbv = pp.tile([P, W], f32, tag="pp")
            nc.tensor.matmul(pbv[:], onesr_t[:], bv_t[:], start=True, stop=True)
            e.bvb_t = pers.tile([P, W], bf16, tag="bvb")
            nc.vector.tensor_copy(e.bvb_t[:], pbv[:])

            # zero tile for output rows beyond a slot's valid-block cap
            e.zt = pers.tile([P, W], f32, tag="zt")
            nc.gpsimd.memset(e.zt[:], 0.0)

            if loop:
                rep_ctx = tc.For_i(
                    0, repeat, 1,
                    hint_engines=(mybir.EngineType.PE,),
                    staggered_reset=staggered,
                )
            else:
                rep_ctx = contextlib.nullcontext(0)
            with rep_ctx:
                for _rep in range(1 if loop else repeat):
                    _emit_iteration(e, slot_caps)
    nc.finalize()
    return nc


def prep_inputs(x, Wq, bq, Wk, bk, Wv, bv, padding_mask):
    """Host-side layout prep + sharding. Returns per-core in_maps."""
    import ml_dtypes
    bf = ml_dtypes.bfloat16
    x = np.asarray(x, dtype=np.float32)
    pad = np.asarray(padding_mask).astype(bool)
    sc = 1.0 / np.sqrt(np.float32(D))
    wqT = np.ascontiguousarray((np.asarray(Wq, np.float32).T * sc).astype(bf))
    wkT = np.ascontiguousarray(np.asarray(Wk, np.float32).T.astype(bf))
    wvT = np.ascontiguousarray(np.asarray(Wv, np.float32).T.astype(bf))
    bq_s = (np.asarray(bq, np.float32) * sc).astype(np.float32)
    bk_a = np.asarray(bk, np.float32)
    bv_a = np.asarray(bv, np.float32).reshape(1, D)
    kbias = np.where(pad, np.float32(NEG), np.float32(0.0)).astype(np.float32)
    qmask = np.where(pad, np.float32(0.0), np.float32(1.0)).astype(np.float32)
    onesr = np.ones((1, P), dtype=np.float32)
    ones = np.ones((P, 1), dtype=np.float32)

    xT = np.ascontiguousarray(x.transpose(0, 2, 1).astype(bf))  # [B, D, S]

    # per-seq valid-block cap from the actual mask (exact for any mask):
    # blocks after the last non-padded position hold only padded positions
    valid = ~pad
    caps = np.zeros(B, dtype=np.int64)
    for b in range(B):
        idx = np.nonzero(valid[b])[0]
        caps[b] = 0 if idx.size == 0 else int(np.ceil((idx[-1] + 1) / P))
    order = np.argsort(-caps, kind="stable")  # descending cap
    # core i runs (slot0 = order[2*N_CORES-1-i] short, slot1 = order[i] long)
    perm = []
    for i in range(N_CORES):
        perm.extend([int(order[B - 1 - i]), int(order[i])])
    slot_caps = (int(caps[order[N_CORES]]), int(caps[order[0]]))

    in_maps = []
    for i in range(N_CORES):
        sel = [perm[2 * i], perm[2 * i + 1]]
        in_maps.append({
            "xT": np.ascontiguousarray(xT[sel]),
            "wqT": wqT, "wkT": wkT, "wvT": wvT,
            "bq": bq_s, "bk": bk_a, "bv": bv_a,
            "kbias": np.ascontiguousarray(kbias[sel]),
            "qmask": np.ascontiguousarray(qmask[sel]),
            "onesr": onesr, "ones": ones,
        })
    return in_maps, perm, slot_caps


_NC_CACHE = {}
STAGGERED = False  # loop back-edge mode (A/B toggle for experiments)


def get_nc(repeat: int = 1, loop: bool = False, slot_caps=(SB, SB)):
    key = (repeat, loop, slot_caps, STAGGERED)
    if key not in _NC_CACHE:
        _NC_CACHE[key] = build_nc(repeat, loop, slot_caps, staggered=STAGGERED)
    return _NC_CACHE[key]


def kernel(x, Wq, bq, Wk, bk, Wv, bv, padding_mask):
    in_maps, perm, slot_caps = prep_inputs(
        x, Wq, bq, Wk, bk, Wv, bv, padding_mask)
    nc = get_nc(1, slot_caps=slot_caps)
    r = run_bass_kernel_spmd(nc, in_maps, list(range(N_CORES)))
    out = np.empty((B, S, D), dtype=np.float32)
    for j, orig in enumerate(perm):
        out[orig] = r.results[j // BPC]["out"][j % BPC]
    return out
